# revision 35
# baseline (speedup 1.0000x reference)
"""DAM module (conv3x3+BN+ReLU -> CAM + PAM attention) on 8 trn2 NeuronCores.

Sharding: core c -> (sample b=c//2, spatial-half h=c%2). Each core computes
the full conv for its sample (bf16 matmuls, fp32 PSUM accum), then CAM and
PAM attention restricted to its half of the output columns. The spatial
order is per-core permuted on the host (own half first) so the compiled
program is identical on every core.

Schedule highlights vs the v0 kernel:
  * BN batch stats: each core contributes OWN-HALF stats only (the pair
    partner covers the other half), so the 8-rank AllGather can be issued
    at the conv midpoint; only a small dummy-matmul bridge remains.
  * PAM inner loop is software-pipelined across j-blocks: energy matmuls
    of block j+1 interleave with apply matmuls of block j, so the PE never
    waits on the tanh of the current group (the old kernel serialized
    E -> tanh -> A per group, paying the full ACT latency 64 times).
  * Conv uses 1024-wide moving operands (16 halo rows cover two adjacent
    512-blocks), halving matmul count.
  * Post-collective latency chain minimized: single-DMA stat gather,
    sum-of-squares on DVE during conv, blockwise ACT relu for feat, and
    the q/k bias drains are first in the DVE queue after the BN chain.
  * feat->ft transposes, v transform and CAM energy accumulation are
    interleaved into the block-0 energy stream to keep the PE dense.

PSUM budget (8 banks): p2k 2x[128,1024] (conv passes, transposes, energy
double-buffer) + pacc 2x[128,512] (dummy bridge, apply accumulators) +
pcam 1x[128,512] (CAM energy, then CAM out) + pmisc 1x[128,512]
(projections, ft/vt transpose batches, attnT).
"""

import sys

for _p in ("/opt/trn_rl_repo",):
    if _p not in sys.path:
        sys.path.insert(0, _p)

from contextlib import ExitStack

import numpy as np
import ml_dtypes

import concourse.bass as bass
import concourse.bacc as bacc
import concourse.tile as tile
from concourse import mybir, masks
from concourse.bass_utils import run_bass_kernel_spmd

BF16NP = ml_dtypes.bfloat16
FP32 = mybir.dt.float32
BF16 = mybir.dt.bfloat16

B, CIN, COUT, H, W = 4, 256, 128, 64, 64
N = H * W          # 4096
NH = N // 2        # 2048 (one spatial half)
CQK = 16
EPS = 1e-5
NCORES = 8
PADH, PADW = 34, 66          # 32+2 halo rows, 64+2 halo cols
XPF = PADH * PADW            # 2244
NBLK = NH // 512             # 4 blocks of 512 per half
NCH = N // 128               # 32 chunks of 128 spatial positions
DUMMY_MMS = 16               # pre-AG bridge
DUMMY2_MMS = 3               # post-AG warm-keeper gated on star8               # fp32 warm-keepers bridging the AllReduce tail


def _build_body(ctx: ExitStack, tc: tile.TileContext, io: dict, ga: float, gp: float):
    nc = tc.nc
    AX = mybir.AxisListType.X
    OP = mybir.AluOpType
    AF = mybir.ActivationFunctionType

    sb = ctx.enter_context(tc.tile_pool(name="sb", bufs=1))
    work = ctx.enter_context(tc.tile_pool(name="work", bufs=1))
    dram = ctx.enter_context(tc.tile_pool(name="dram", bufs=1, space="DRAM"))
    p2k = ctx.enter_context(tc.tile_pool(name="p2k", bufs=2, space="PSUM"))
    pacc = ctx.enter_context(tc.tile_pool(name="pacc", bufs=2, space="PSUM"))
    pcam = ctx.enter_context(tc.tile_pool(name="pcam", bufs=1, space="PSUM"))
    pmisc = ctx.enter_context(tc.tile_pool(name="pmisc", bufs=1, space="PSUM"))

    # ---- x slots (2 halves x 2 cin chunks), own half first ----
    x_sb = []
    HALF1 = 17 * PADW
    for i in range(4):
        t = sb.tile([128, XPF], BF16, tag=f"xp{i}")
        cuts = [0, 9 * PADW, HALF1, 26 * PADW, XPF]
        for c0, c1 in zip(cuts[:-1], cuts[1:]):
            nc.sync.dma_start(out=t[:, c0:c1], in_=io["xp"][i][:, c0:c1])
        x_sb.append(t)

    # ---- load constants / weights (cin-chunk-0 taps first) ----
    cw_sb = sb.tile([128, 18 * 128], BF16, tag="cw")
    for i in range(4):
        lo, hi = i * 576, min((i + 1) * 576, 18 * 128)
        nc.sync.dma_start(out=cw_sb[:, lo:hi], in_=io["cw"][:, lo:hi])
    qwt_sb = sb.tile([128, CQK], BF16, tag="qwt")
    nc.sync.dma_start(out=qwt_sb[:], in_=io["qwt"])
    kwt_sb = sb.tile([128, CQK], BF16, tag="kwt")
    nc.sync.dma_start(out=kwt_sb[:], in_=io["kwt"])
    vwt_sb = sb.tile([128, 128], BF16, tag="vwt")
    nc.sync.dma_start(out=vwt_sb[:], in_=io["vwt"])
    # q/k biases duplicated at partition strips 0 and 32 (the energy
    # matmuls run 2x row-tiled, so k/q live in two partition strips)
    qb_sb = sb.tile([64, 1], FP32, tag="qb")
    nc.sync.dma_start(out=qb_sb[0:CQK], in_=io["qb"])
    nc.sync.dma_start(out=qb_sb[32:32 + CQK], in_=io["qb"])
    kb_sb = sb.tile([64, 1], FP32, tag="kb")
    nc.sync.dma_start(out=kb_sb[0:CQK], in_=io["kb"])
    nc.sync.dma_start(out=kb_sb[32:32 + CQK], in_=io["kb"])
    bng_sb = sb.tile([128, 1], FP32, tag="bng")
    nc.sync.dma_start(out=bng_sb[:], in_=io["bng"])
    bnb_sb = sb.tile([128, 1], FP32, tag="bnb")
    nc.sync.dma_start(out=bnb_sb[:], in_=io["bnb"])
    # v bias broadcast across partitions (DMA partition-step-0 replication)
    vbb = sb.tile([128, 128], FP32, tag="vbb")
    vb_ap = io["vb"]
    nc.sync.dma_start(
        out=vbb[:],
        in_=bass.AP(tensor=vb_ap.tensor, offset=vb_ap.offset, ap=[[0, 128], [1, 128]]),
    )
    ident = sb.tile([128, 128], BF16, tag="ident")
    masks.make_identity(nc, ident[:])

    # ---- conv3x3: y[cout, n] in bf16; stats (DVE) for OWN half only ----
    yb = sb.tile([128, N], BF16, tag="yb")
    sums4 = sb.tile([128, 4], FP32, tag="sums4")
    sq4 = sb.tile([128, 4], FP32, tag="sq4")
    dumsrc = sb.tile([128, 512], FP32, tag="dumsrc")

    # 4 passes of 2 blocks each; weight-outer so each pass does 18
    # LDWEIGHTS and 36 back-to-back matmuls into a [128,1024] accumulator.
    for p in range(4):
        yp = p2k.tile([128, 1024], FP32, tag="big")
        m = 0
        for k in range(2):
            for di in range(3):
                for dj in range(3):
                    wi = 9 * k + di * 3 + dj
                    for r in range(2):
                        blk = 2 * p + r          # global 512-block index
                        s, j = blk // NBLK, blk % NBLK
                        xv = x_sb[s * 2 + k][:].rearrange(
                            "p (r w) -> p r w", w=PADW)
                        nc.tensor.matmul(
                            yp[:, r * 512:(r + 1) * 512],
                            cw_sb[:, wi * 128:(wi + 1) * 128],
                            xv[:, 8 * j + di: 8 * j + di + 8, dj: dj + 64],
                            start=(m < 2),
                            stop=(m >= 34),
                            skip_group_check=True,
                        )
                        m += 1
        for r in range(2):
            t = 2 * p + r
            ypr = yp[:, r * 512:(r + 1) * 512]
            if p < 2:
                # own-half stats only; sum on DVE, sum-of-squares on ACT
                nc.vector.reduce_sum(out=sums4[:, t: t + 1], in_=ypr, axis=AX)
                scr = work.tile([128, 512], BF16, tag="scr", bufs=2)
                nc.scalar.activation(out=scr[:], in_=ypr, func=AF.Square,
                                     accum_out=sq4[:, t: t + 1])
            nc.vector.tensor_copy(out=yb[:, t * 512:(t + 1) * 512], in_=ypr)
            if p == 0 and r == 0:
                nc.vector.tensor_copy(out=dumsrc[:], in_=ypr)

        if p == 1:
            # ---- BN stats AllGather across all 8 cores (own halves) ----
            st = sb.tile([128, 2], FP32, tag="st")
            nc.vector.reduce_sum(out=st[:, 0:1], in_=sums4[:], axis=AX)
            nc.vector.reduce_sum(out=st[:, 1:2], in_=sq4[:], axis=AX)
            cc_in = dram.tile([128, 2], FP32, tag="ccin")
            cc_out = dram.tile([1024, 2], FP32, tag="ccout")
            nc.sync.dma_start(out=cc_in[:], in_=st[:])
            nc.gpsimd.collective_compute(
                "AllGather", OP.bypass, ins=[cc_in.opt()], outs=[cc_out.opt()],
                replica_groups=[[0, 1, 2, 3, 4, 5, 6, 7]],
            )

    # Warm-keeper fp32 matmuls bridging the residual collective latency.
    dummy = pacc.tile([128, 512], FP32, tag="acc")
    for i in range(DUMMY_MMS):
        nc.tensor.matmul(
            dummy[:], dumsrc[:, 0:128], dumsrc[:],
            start=(i == 0), stop=(i == DUMMY_MMS - 1),
        )

    # ---- gather the 8 contributions (single strided DMA) and reduce ----
    star8 = sb.tile([128, 16], FP32, tag="star8")
    cco = cc_out[:]
    nc.sync.dma_start(
        out=star8[:],
        in_=bass.AP(tensor=cco.tensor, offset=cco.offset,
                    ap=[[2, 128], [256, 8], [1, 2]]),
    )
    # Post-AG warm-keeper: fp32 matmuls gated on the gathered stats keep
    # the PE at full clock through the BN coefficient chain, so the first
    # projection/energy matmuls run warm.
    dummy2 = pacc.tile([128, 512], FP32, tag="acc")
    for i in range(DUMMY2_MMS):
        nc.tensor.matmul(
            dummy2[0:16, :], star8[:], dumsrc[:],
            start=(i == 0), stop=(i == DUMMY2_MMS - 1),
        )

    star = sb.tile([128, 2], FP32, tag="star")
    nc.vector.reduce_sum(
        out=star[:].rearrange("p (t o) -> p t o", o=1),
        in_=star8[:].rearrange("p (i t) -> p t i", t=2),
        axis=AX,
    )

    # ---- BN coefficients (shortest serial chain):
    # feat = relu(a*y - nb) with a = gamma/std, nb = mean*a - beta ----
    inv_n = 1.0 / float(B * N)
    star_n = sb.tile([128, 2], FP32, tag="star_n")
    nc.vector.tensor_scalar_mul(out=star_n[:], in0=star[:], scalar1=inv_n)
    mean = star_n[:, 0:1]
    var = sb.tile([128, 1], FP32, tag="var")
    mean2 = sb.tile([128, 1], FP32, tag="mean2")
    nc.vector.tensor_mul(out=mean2[:], in0=mean, in1=mean)
    nc.vector.tensor_sub(out=var[:], in0=star_n[:, 1:2], in1=mean2[:])
    eps_sb = sb.tile([128, 1], FP32, tag="eps")
    nc.vector.memset(eps_sb[:], EPS)
    std = sb.tile([128, 1], FP32, tag="std")
    nc.scalar.activation(out=std[:], in_=var[:], func=AF.Sqrt, bias=eps_sb[:])
    rstd = sb.tile([128, 1], FP32, tag="rstd")
    nc.vector.reciprocal(out=rstd[:], in_=std[:])
    acoef = sb.tile([128, 1], FP32, tag="acoef")
    nc.vector.tensor_mul(out=acoef[:], in0=bng_sb[:], in1=rstd[:])
    nbcoef = sb.tile([128, 1], FP32, tag="nbcoef")
    nc.vector.scalar_tensor_tensor(
        out=nbcoef[:], in0=mean, scalar=acoef[:], in1=bnb_sb[:],
        op0=OP.mult, op1=OP.subtract)
    bcoef = sb.tile([128, 1], FP32, tag="bcoef")
    nc.vector.tensor_scalar_mul(out=bcoef[:], in0=nbcoef[:], scalar1=-1.0)
    zcol = sb.tile([128, 1], FP32, tag="zcol")
    nc.vector.memset(zcol[:], 0.0)

    # ---- feat = relu(a*y + b) on DVE, blockwise so the projections and
    # the first energies can chase the blocks (ACT stays free for tanh) ----
    feat = sb.tile([128, N], BF16, tag="feat")
    tmp1 = sb.tile([128, N], BF16, tag="tmp1")
    q_sb = sb.tile([64, NH], BF16, tag="q")
    k_sb = sb.tile([64, N // 2], BF16, tag="k")
    FP8 = mybir.dt.float8e4
    ft = sb.tile([128, N], BF16, tag="ft")
    vt = sb.tile([128, N], FP8, tag="vt")
    at_a = sb.tile([128, 16 * 1024], FP8, tag="at0")
    at_b = sb.tile([128, 16 * 1024], FP8, tag="at1")
    at_buf = [at_a, at_b]
    attnT = sb.tile([128, 128], BF16, tag="attnT")
    en_sb = sb.tile([128, 128], FP32, tag="en_sb")
    out_sb = sb.tile([128, NH], FP32, tag="osb")

    def emit_energy(j, hg):
        # chunk 2*hg at row-tile 0, chunk 2*hg+1 at row-tile 32: the two
        # matmuls run concurrently in separate 32-row strips; tanh drains
        # the tile straight from PSUM into the fp8 at buffer.
        ep = p2k.tile([128, 1024], FP32, tag="big")
        for r in range(2):
            nc.tensor.matmul(
                ep[:, r * 512:(r + 1) * 512],
                k_sb[32 * r:32 * r + CQK, hg * 128:(hg + 1) * 128],
                q_sb[32 * r:32 * r + CQK, j * 512:(j + 1) * 512],
                start=True, stop=True, skip_group_check=True,
                tile_position=(32 * r, 0),
            )
        nc.scalar.activation(
            out=at_buf[j % 2][:, hg * 1024:(hg + 1) * 1024],
            in_=ep[:], func=AF.Tanh)

    def emit_tp(bch):
        # transpose 4 chunks (ft) + v transform for the same 4 chunks
        tp = p2k.tile([128, 1024], FP32, tag="big")
        for u in range(4):
            t = 4 * bch + u
            nc.tensor.matmul(
                tp[:, u * 128:(u + 1) * 128],
                feat[:, t * 128:(t + 1) * 128],
                ident[:],
                start=True, stop=True, skip_group_check=True,
            )
        for u in range(4):
            t = 4 * bch + u
            nc.tensor.matmul(
                tp[:, (4 + u) * 128:(5 + u) * 128],
                feat[:, t * 128:(t + 1) * 128],
                vwt_sb[:],
                start=True, stop=True, skip_group_check=True,
            )
        nc.vector.tensor_copy(out=ft[:, bch * 512:(bch + 1) * 512],
                              in_=tp[:, 0:512])
        nc.vector.tensor_add(
            out=vt[:, bch * 512:(bch + 1) * 512], in0=tp[:, 512:1024],
            in1=bass.AP(tensor=vbb[:].tensor, offset=vbb[:].offset,
                        ap=[vbb[:].ap[0], [0, 4], [1, 128]]))

    nbcoef_bc = bass.AP(tensor=nbcoef[:].tensor, offset=nbcoef[:].offset,
                        ap=[nbcoef[:].ap[0], [0, 512]])
    for j in range(8):
        if j < NBLK:
            # own half on ACT: runs before the tanh stream exists, so the
            # first energies are unblocked ~2us after the coefficients
            nc.scalar.activation(
                out=feat[:, j * 512:(j + 1) * 512],
                in_=yb[:, j * 512:(j + 1) * 512], func=AF.Relu,
                bias=bcoef[:], scale=acoef[:])
            nc.vector.scalar_tensor_tensor(
                out=tmp1[:, j * 512:(j + 1) * 512],
                in0=yb[:, j * 512:(j + 1) * 512], scalar=acoef[:],
                in1=nbcoef_bc, op0=OP.mult, op1=OP.subtract)
        else:
            # other half on DVE, in parallel with the ACT relu blocks
            nc.vector.scalar_tensor_tensor(
                out=tmp1[:, j * 512:(j + 1) * 512],
                in0=yb[:, j * 512:(j + 1) * 512], scalar=acoef[:],
                in1=nbcoef_bc, op0=OP.mult, op1=OP.subtract)
            nc.vector.tensor_scalar_max(
                out=feat[:, j * 512:(j + 1) * 512],
                in0=tmp1[:, j * 512:(j + 1) * 512], scalar1=0.0)
        fj = feat[:, j * 512:(j + 1) * 512]
        # k chunks go to partition strip 32*(t%2), column u = t//2; q is
        # replicated at both strips.  Column-tiled pairs run concurrently.
        kp = pmisc.tile([64, 256], FP32, tag="m")
        for r in range(2):
            rhs = bass.AP(tensor=fj.tensor, offset=fj.offset + r * 128,
                          ap=[fj.ap[0], [256, 2], [1, 128]])
            nc.tensor.matmul(kp[32 * r:32 * r + CQK, :], kwt_sb[:], rhs,
                             start=True, stop=True, skip_group_check=True,
                             tile_position=(0, 32 * r))
            nc.vector.tensor_scalar_add(
                out=k_sb[32 * r:32 * r + CQK, j * 256:(j + 1) * 256],
                in0=kp[32 * r:32 * r + CQK, :],
                scalar1=kb_sb[32 * r:32 * r + CQK, :])
        if j < NBLK:
            qp = pmisc.tile([64, 512], FP32, tag="m")
            for r in range(2):
                nc.tensor.matmul(qp[32 * r:32 * r + CQK, :], qwt_sb[:], fj,
                                 start=True, stop=True, skip_group_check=True,
                                 tile_position=(0, 32 * r))
                nc.vector.tensor_scalar_add(
                    out=q_sb[32 * r:32 * r + CQK, j * 512:(j + 1) * 512],
                    in0=qp[32 * r:32 * r + CQK, :],
                    scalar1=qb_sb[32 * r:32 * r + CQK, :])
        # block-0 energies + transposes chase the feat/projection blocks
        emit_energy(0, 2 * j)
        emit_energy(0, 2 * j + 1)
        emit_tp(j)

    # out_sb = 3*relu(tmp1), late in the DVE queue
    for j in range(NBLK):
        nc.vector.tensor_scalar(
            out=out_sb[:, j * 512:(j + 1) * 512],
            in0=tmp1[:, j * 512:(j + 1) * 512],
            scalar1=0.0, scalar2=3.0, op0=OP.max, op1=OP.mult)

    # Steady: energies of j+1 + one fp8-DoubleRow apply of j per group
    # (apply g consumes exactly tanh group g of the previous block); CAM
    # energy accumulation rides in steady-0.
    en_ps = pcam.tile([128, 512], FP32, tag="cam")
    for j in range(NBLK):
        ops = pacc.tile([128, 512], FP32, tag="acc")
        vtr = vt[:].rearrange("p (t c) -> p t c", c=128)
        atr = at_buf[j % 2][:].rearrange("p (t m) -> p t m", m=512)
        for hg in range(16):
            if j + 1 < NBLK:
                emit_energy(j + 1, hg)
            if j == 0:
                for u in range(2):
                    t = 2 * hg + u
                    nc.tensor.matmul(
                        en_ps[:, 0:128],
                        ft[:, t * 128:(t + 1) * 128],
                        ft[:, t * 128:(t + 1) * 128],
                        start=(t == 0), stop=(t == NCH - 1),
                        skip_group_check=True,
                    )
            nc.tensor.matmul(
                ops[:],
                vtr[:, 2 * hg:2 * hg + 2, :],
                atr[:, 2 * hg:2 * hg + 2, :],
                start=(hg == 0), stop=(hg == 15), skip_group_check=True,
                perf_mode=mybir.MatmulPerfMode.DoubleRow,
            )
        if j == 0:
            # CAM attention map: attn = tanh(max(en) - en), then transpose
            nc.vector.tensor_copy(out=en_sb[:], in_=en_ps[:, 0:128])
            mx = sb.tile([128, 1], FP32, tag="mx")
            nc.vector.reduce_max(out=mx[:], in_=en_sb[:], axis=AX)
            en_new = sb.tile([128, 128], FP32, tag="en_new")
            nc.vector.tensor_scalar(
                out=en_new[:], in0=en_sb[:], scalar1=mx[:], scalar2=-1.0,
                op0=OP.subtract, op1=OP.mult,
            )
            attn = sb.tile([128, 128], BF16, tag="attn")
            nc.scalar.activation(out=attn[:], in_=en_new[:], func=AF.Tanh)
            atp = pmisc.tile([128, 512], FP32, tag="m")
            nc.tensor.matmul(atp[:, 0:128], attn[:], ident[:],
                             start=True, stop=True, skip_group_check=True)
            nc.vector.tensor_copy(out=attnT[:], in_=atp[:, 0:128])
        # CAM term for block j, then the gamma-weighted accumulation + DMA
        cps = pcam.tile([128, 512], FP32, tag="cam")
        nc.tensor.matmul(cps[:], attnT[:], feat[:, j * 512:(j + 1) * 512],
                         start=True, stop=True)
        nc.vector.scalar_tensor_tensor(
            out=out_sb[:, j * 512:(j + 1) * 512],
            in0=ops[:], scalar=gp, in1=out_sb[:, j * 512:(j + 1) * 512],
            op0=OP.mult, op1=OP.add)
        nc.vector.scalar_tensor_tensor(
            out=out_sb[:, j * 512:(j + 1) * 512],
            in0=cps[:], scalar=ga, in1=out_sb[:, j * 512:(j + 1) * 512],
            op0=OP.mult, op1=OP.add)
        nc.sync.dma_start(out=io["out"][:, j * 512:(j + 1) * 512],
                          in_=out_sb[:, j * 512:(j + 1) * 512])


def build_nc(ga: float, gp: float):
    nc = bacc.Bacc("TRN2", target_bir_lowering=False, debug=False,
                   num_devices=NCORES)
    io = {
        "xp": nc.dram_tensor("xp", [4, 128, XPF], BF16, kind="ExternalInput").ap(),
        "cw": nc.dram_tensor("cw", [128, 18 * 128], BF16, kind="ExternalInput").ap(),
        "qwt": nc.dram_tensor("qwt", [128, CQK], BF16, kind="ExternalInput").ap(),
        "kwt": nc.dram_tensor("kwt", [128, CQK], BF16, kind="ExternalInput").ap(),
        "vwt": nc.dram_tensor("vwt", [128, 128], BF16, kind="ExternalInput").ap(),
        "qb": nc.dram_tensor("qb", [CQK, 1], FP32, kind="ExternalInput").ap(),
        "kb": nc.dram_tensor("kb", [CQK, 1], FP32, kind="ExternalInput").ap(),
        "vb": nc.dram_tensor("vb", [1, 128], FP32, kind="ExternalInput").ap(),
        "bng": nc.dram_tensor("bng", [128, 1], FP32, kind="ExternalInput").ap(),
        "bnb": nc.dram_tensor("bnb", [128, 1], FP32, kind="ExternalInput").ap(),
        "out": nc.dram_tensor("out", [128, NH], FP32, kind="ExternalOutput").ap(),
    }
    with tile.TileContext(nc) as tc, ExitStack() as ctx:
        _build_body(ctx, tc, io, ga, gp)
    nc.compile()
    return nc


def make_in_maps(x, conv_w, bn_gamma, bn_beta, q_w, q_b, k_w, k_b, v_w, v_b):
    x = np.asarray(x, np.float32)
    conv_w = np.asarray(conv_w, np.float32)

    xpad = np.zeros((B, CIN, H + 2, W + 2), np.float32)
    xpad[:, :, 1:H + 1, 1:W + 1] = x

    cw = np.empty((128, 18 * 128), np.float32)
    for di in range(3):
        for dj in range(3):
            for k in range(2):
                wi = 9 * k + di * 3 + dj
                cw[:, wi * 128:(wi + 1) * 128] = conv_w[:, k * 128:(k + 1) * 128, di, dj].T
    shared = {
        "cw": cw.astype(BF16NP),
        "qwt": np.ascontiguousarray(np.asarray(q_w, np.float32).T).astype(BF16NP),
        "kwt": np.ascontiguousarray(np.asarray(k_w, np.float32).T).astype(BF16NP),
        "vwt": np.ascontiguousarray(np.asarray(v_w, np.float32).T).astype(BF16NP),
        "qb": np.asarray(q_b, np.float32).reshape(CQK, 1),
        "kb": np.asarray(k_b, np.float32).reshape(CQK, 1),
        "vb": np.asarray(v_b, np.float32).reshape(1, 128),
        "bng": np.asarray(bn_gamma, np.float32).reshape(128, 1),
        "bnb": np.asarray(bn_beta, np.float32).reshape(128, 1),
    }

    in_maps = []
    for c in range(NCORES):
        b, h = c // 2, c % 2
        xp = np.empty((4, 128, XPF), np.float32)
        for s, half in enumerate((h, 1 - h)):
            blk = xpad[b, :, 32 * half:32 * half + PADH, :]  # [256, 34, 66]
            for k in range(2):
                xp[s * 2 + k] = blk[k * 128:(k + 1) * 128].reshape(128, XPF)
        m = dict(shared)
        m["xp"] = xp.astype(BF16NP)
        in_maps.append(m)
    return in_maps


_NC_CACHE: dict = {}


def kernel(x, conv_w, bn_gamma, bn_beta, q_w, q_b, k_w, k_b, v_w, v_b,
           gamma_ca, gamma_pa):
    ga = float(np.asarray(gamma_ca).reshape(-1)[0])
    gp = float(np.asarray(gamma_pa).reshape(-1)[0])
    key = (ga, gp)
    if key not in _NC_CACHE:
        _NC_CACHE[key] = build_nc(ga, gp)
    nc = _NC_CACHE[key]

    in_maps = make_in_maps(x, conv_w, bn_gamma, bn_beta,
                           q_w, q_b, k_w, k_b, v_w, v_b)
    res = run_bass_kernel_spmd(nc, in_maps, core_ids=list(range(NCORES)))

    out = np.empty((B, COUT, H, W), np.float32)
    for c in range(NCORES):
        b, h = c // 2, c % 2
        out[b, :, 32 * h:32 * h + 32, :] = \
            res.results[c]["out"].reshape(COUT, 32, W)
    return out
